# revision 52
# baseline (speedup 1.0000x reference)
"""Trainium2 Bass kernel for nn_Explainer segment_reduce (cdist + bidirectional
segment max/mean) on 8 NeuronCores.

Math (reference):
    ef_n = (h[ne0] + h[ne1])/2, ef_l = (h[le0] + h[le1])/2
    M = -cdist(ef_n, ef_l)                      # [En, El]
    out_n = seg_mean_rows(seg_max_cols(M))      # [Gn, Gl]
    out_l = seg_mean_cols(seg_max_rows(M))      # [Gn, Gl]
    out = (out_n + out_l)/2

Device computes strip = 2*u_n.u_l - |u_l|^2 - |u_n|^2 = -|u_n - u_l|^2 = -4d^2:
the dot term via fp8e4 DoubleRow matmuls (K=256 in one instruction), -|u_l|^2
via a K=1 ones x (-bl2) fp16 matmul (spread over PE quadrant rows), -|u_n|^2
as the per-partition ACT bias during the psum->fp16 copy. All segment
reductions are then plain MAX; host maps back via d = 0.5*sqrt(-v).

Sharding: core c owns node segments [8c, 8c+8) in per-segment lane bands
(segment s -> lanes [B_s, B_s+L_s), row-tiles t in [0, nrt)); dummy slots
duplicate the segment's first row (can't win a max, no masking needed).
Label columns replicated; each segment padded to a multiple of W=8 with
duplicate edges (can't win a max either).

Per tile: PE fills psum groups; ACT converts fp32 psum -> fp16 strip with the
an2 bias; DVE does the W-block max tree (row side -> [P, B] block maxes) and
the running col max across tiles (col side). Host: fold blocks per label
segment + sqrt + means; band-collapse the col accumulator + sqrt + masked
means; assemble [64, 64].
"""
import numpy as np

import concourse.bacc as bacc
import concourse.tile as tile
import concourse.mybir as mybir
from concourse.bass_utils import run_bass_kernel_spmd

P = 128
N_CORES = 8
GN = GL = 64
D = 256
W = 8                      # label block width for the row-side max tree
F16 = mybir.dt.float16
F32 = mybir.dt.float32
F8 = mybir.dt.float8e4

_prog_cache = {}


def _groups(C):
    """psum column groups of 2048 (4 banks) + optional 512/1024/1536 rem."""
    gs = [(i * 2048, 2048) for i in range(C // 2048)]
    if C % 2048:
        gs.append((C - C % 2048, C % 2048))
    return gs


def _build(nrt: int, C: int, use_fp8: bool = False, C_real: int = 0):
    B = C // W
    C_real = C_real or C
    B_real = C_real // W
    groups = _groups(C)
    FD = F8 if use_fp8 else F16

    nc = bacc.Bacc("TRN2", target_bir_lowering=False, debug=False,
                   num_devices=N_CORES)
    ulT_in = nc.dram_tensor("ulT", [P, 2 * C], FD, kind="ExternalInput")
    unT_in = nc.dram_tensor("unT", [P, nrt * 2 * P], FD, kind="ExternalInput")
    bl2_in = nc.dram_tensor("bl2c", [4, C], F16, kind="ExternalInput")
    an2_in = nc.dram_tensor("an2q", [P, nrt], F32, kind="ExternalInput")
    rowout = nc.dram_tensor("rowout", [P, nrt * B], F16, kind="ExternalOutput")
    collout = nc.dram_tensor("collout", [P, C], F16, kind="ExternalOutput")

    with tile.TileContext(nc) as tc:
        with (
            tc.tile_pool(name="persist", bufs=1) as pp,
            tc.tile_pool(name="strip", bufs=4) as sp,
            tc.tile_pool(name="s1", bufs=3) as s1p,
            tc.tile_pool(name="row", bufs=4) as rp,
        ):
            u_lT = [pp.tile([P, 2, w], FD, tag=f"u_lT{gi}",
                            name=f"u_lT{gi}")
                    for gi, (g0, w) in enumerate(groups)]
            u_nT = pp.tile([P, nrt, 2, P], FD, tag="u_nT")
            bl2rep = pp.tile([P, C], F16, tag="bl2rep")
            onesrep = pp.tile([P, P], F16, tag="onesrep")
            an2q = pp.tile([P, nrt], F32, tag="an2q")
            coll = pp.tile([P, C], F16, tag="coll")

            nc.gpsimd.memset(onesrep[:], 1.0)
            # issue order: tile-0/group-0 dependencies only; remaining label
            # groups are prefetched inside the tile-0 loop (the DMA sem is
            # cumulative per queue, so anything emitted before the first
            # matmul delays it)
            nc.sync.dma_start(u_nT[:].rearrange("p t k q -> p (t k q)"),
                              unT_in[:])
            g0, w = groups[0]
            for k in range(2):
                nc.sync.dma_start(u_lT[0][:, k, :],
                                  ulT_in[:, k * C + g0:k * C + g0 + w])
            nc.sync.dma_start(bl2rep[0:97:32, :], bl2_in[:])
            nc.sync.dma_start(an2q[:], an2_in[:])

            collv = coll[:].rearrange("p (b w) -> p b w", w=W)
            with tc.tile_pool(name="ps", bufs=2, space="PSUM") as pg:
                for t in range(nrt):
                    strip = sp.tile([P, B, W], F16, tag="strip")
                    stripf = strip[:].rearrange("p b w -> p (b w)")
                    for gi, (g0, w) in enumerate(groups):
                        if t == 0 and gi + 1 < len(groups):
                            # prefetch next label group during this one's work
                            gn, wn = groups[gi + 1]
                            for k in range(2):
                                nc.sync.dma_start(
                                    u_lT[gi + 1][:, k, :],
                                    ulT_in[:, k * C + gn:k * C + gn + wn])
                        ptf = pg.tile([P, 2048], F32, tag="dot")
                        pt = ptf[:, :w]
                        nchunk = w // 512
                        if use_fp8:
                            for j in range(nchunk):
                                nc.tensor.matmul(
                                    pt[:, j * 512:(j + 1) * 512],
                                    u_nT[:, t, :, :],
                                    u_lT[gi][:, :, j * 512:(j + 1) * 512],
                                    start=True, stop=False,
                                    perf_mode=mybir.MatmulPerfMode.DoubleRow)
                        else:
                            for k in range(2):
                                for j in range(nchunk):
                                    nc.tensor.matmul(
                                        pt[:, j * 512:(j + 1) * 512],
                                        u_nT[:, t, k, :],
                                        u_lT[gi][:, k, j * 512:(j + 1) * 512],
                                        start=(k == 0), stop=False)
                        for j in range(nchunk):
                            pb = 32 * (j % 4)
                            nc.tensor.matmul(
                                pt[:, j * 512:(j + 1) * 512],
                                onesrep[pb:pb + 1, :],
                                bl2rep[pb:pb + 1,
                                       g0 + j * 512:g0 + (j + 1) * 512],
                                start=False, stop=True,
                                tile_position=(pb, 0))
                        dst = coll[:, g0:g0 + w] if t == 0 else stripf[:, g0:g0 + w]
                        nc.scalar.activation(
                            dst, pt[:], mybir.ActivationFunctionType.Identity,
                            bias=an2q[:, t:t + 1], scale=1.0)
                        # col side: running max, emitted per group so DVE
                        # tracks the conversions (t=0 wrote coll directly)
                        we = min(w, C_real - g0)       # skip pad-only cols
                        if t > 0 and we > 0:
                            nc.vector.tensor_max(coll[:, g0:g0 + we],
                                                 coll[:, g0:g0 + we],
                                                 stripf[:, g0:g0 + we])
                            if t == nrt - 1:
                                nc.sync.dma_start(collout[:, g0:g0 + we],
                                                  coll[:, g0:g0 + we])
                    # row side: W-block max tree -> [P, B_real]
                    src = collv if t == 0 else strip[:]
                    s1 = s1p.tile([P, B_real, 4], F16, tag="s1")
                    nc.vector.tensor_max(s1[:], src[:, :B_real, 0:4],
                                         src[:, :B_real, 4:8])
                    nc.vector.tensor_max(s1[:, :, 0:2], s1[:, :, 0:2],
                                         s1[:, :, 2:4])
                    rst = rp.tile([P, B_real], F16, tag="rst")
                    nc.vector.tensor_max(rst[:], s1[:, :, 0], s1[:, :, 1])
                    nc.sync.dma_start(rowout[:, t * B:t * B + B_real], rst[:])

    nc.compile()
    return nc


def _get_program(nrt, C, use_fp8, C_real):
    key = (nrt, C, use_fp8, C_real)
    if key not in _prog_cache:
        _prog_cache[key] = _build(nrt, C, use_fp8, C_real)
    return _prog_cache[key]


def _band_layout(sizes, nrt):
    """Lane bands: segment s gets L_s = ceil(size_s/nrt) lanes."""
    L = [-(-int(s) // nrt) if s > 0 else 0 for s in sizes]
    B = np.concatenate([[0], np.cumsum(L)]).astype(np.int64)
    return B, L


def kernel(h, node_edge, node_batch, label_edge, label_batch):
    h = np.asarray(h)
    ne = np.asarray(node_edge).astype(np.int64)
    nb = np.asarray(node_batch).astype(np.int64)
    le = np.asarray(label_edge).astype(np.int64)
    lb = np.asarray(label_batch).astype(np.int64)
    use_fp8 = True
    fdt = mybir.dt.np(F8 if use_fp8 else F16)

    cn = np.bincount(nb, minlength=GN).astype(np.int64)
    cl = np.bincount(lb, minlength=GL).astype(np.int64)
    nb_off = np.concatenate([[0], np.cumsum(cn)])
    lb_off = np.concatenate([[0], np.cumsum(cl)])

    # ---- label columns: each segment padded to a multiple of W with
    # duplicate edges; then global pad to a multiple of 512 with col 0 dups
    bg = -(-cl // W)                       # blocks per segment
    b_off = np.concatenate([[0], np.cumsum(bg)])
    B_real = int(b_off[-1])
    C = -(-(B_real * W) // 512) * 512
    B = C // W

    col_edge = np.zeros(C, np.int64)
    for g in range(GL):
        n_g = int(cl[g])
        if n_g == 0:
            continue
        width = int(bg[g]) * W
        k = np.arange(width)
        col_edge[b_off[g] * W + k] = lb_off[g] + k % n_g

    hf = h.astype(np.float32)
    u_l = hf[le[0][col_edge]] + hf[le[1][col_edge]]            # [C, 256] fp32
    bq = u_l.astype(fdt)                                       # quantized b
    bl2 = (bq.astype(np.float32) ** 2).sum(axis=1)             # |b|^2
    ulT = np.ascontiguousarray(
        bq.T.reshape(2, P, C).transpose(1, 0, 2).reshape(P, 2 * C))
    bl2_rep = np.ascontiguousarray(np.broadcast_to(
        (-bl2).astype(np.float16)[None, :], (4, C)))

    # ---- node rows: per-core lane bands over 8 segments; dummy slots
    # duplicate the segment's first row
    core_sizes = cn.reshape(N_CORES, 8)
    nrt = max(1, int(-(-core_sizes.sum(1).max() // P)))
    while max(sum(-(-int(s) // nrt) for s in core_sizes[c] if s > 0)
              for c in range(N_CORES)) > P:
        nrt += 1
    nrows = nrt * P

    in_maps = []
    band_info = []
    for c in range(N_CORES):
        Bo, L = _band_layout(core_sizes[c], nrt)
        assert Bo[-1] <= P
        slot = np.zeros(nrows, np.int64)
        # fallback row for fully-unused lanes (any valid index)
        slot[:] = min(int(nb_off[8 * c]), ne.shape[1] - 1)
        for s in range(8):
            g = 8 * c + s
            n_g = int(cn[g])
            if n_g == 0:
                continue
            lanes_all = np.arange(Bo[s], Bo[s + 1])
            for tt in range(nrt):
                slot[tt * P + lanes_all] = nb_off[g]   # seg dup default
            j = np.arange(n_g)
            lanes = Bo[s] + j // nrt
            ts = j % nrt
            slot[ts * P + lanes] = nb_off[g] + j
        u_n = hf[ne[0][slot]] + hf[ne[1][slot]]                 # [nrows, 256]
        aq = (2.0 * u_n).astype(fdt)                            # quantized a
        an2 = ((aq.astype(np.float32) ** 2).sum(axis=1) * 0.25)
        # unT layout: [p(K%128), t, k, row] ; row r of tile t = aq[t*P + r]
        a = aq.reshape(nrt, P, 2, P)         # [t, row, k, p]
        unT = np.ascontiguousarray(a.transpose(3, 0, 2, 1).reshape(P, -1))
        an2q = np.ascontiguousarray((-an2).astype(np.float32)
                                    .reshape(nrt, P).T)
        in_maps.append({
            "ulT": ulT,
            "unT": unT,
            "bl2c": bl2_rep,
            "an2q": an2q,
        })
        band_info.append((Bo, L))

    nc = _get_program(nrt, C, use_fp8, B_real * W)
    res = run_bass_kernel_spmd(nc, in_maps, core_ids=list(range(N_CORES)))

    # ---- host unpack -----------------------------------------------------
    out_n = np.zeros((GN, GL), np.float64)
    out_l = np.zeros((GN, GL), np.float64)
    col_w = np.zeros(C, np.float64)
    for g in range(GL):
        col_w[b_off[g] * W:b_off[g] * W + int(cl[g])] = 1.0
    ridx = (b_off[:-1]).clip(0, max(B_real - 1, 0))
    cidx = (b_off[:-1] * W).clip(0, max(B_real * W - 1, 0))
    for c in range(N_CORES):
        r = res.results[c]
        rowe = r["rowout"].astype(np.float64).reshape(P, nrt, B)
        colle = r["collout"].astype(np.float64)                 # [128, C]
        Bo, L = band_info[c]
        for s in range(8):
            g = 8 * c + s
            n_g = int(cn[g])
            if n_g == 0:
                continue
            j = np.arange(n_g)
            lanes = Bo[s] + j // nrt
            ts = j % nrt
            blk = rowe[lanes, ts, :]                            # [n_g, B]
            segmax = np.maximum.reduceat(blk[:, :B_real], ridx, axis=1)
            d = 0.5 * np.sqrt(np.maximum(-segmax, 0.0))
            row_mean = -d.mean(axis=0)
            row_mean[cl == 0] = 0.0
            out_n[g] = row_mean

            ecol = colle[Bo[s]:Bo[s] + L[s], :].max(axis=0)     # [C]
            dcol = 0.5 * np.sqrt(np.maximum(-ecol, 0.0))
            sums = np.add.reduceat((dcol * col_w)[:B_real * W], cidx)
            col_mean = -(sums / np.maximum(cl, 1))
            col_mean[cl == 0] = 0.0
            out_l[g] = col_mean

    return ((out_n + out_l) * 0.5).astype(np.float32)


# revision 53
# speedup vs baseline: 1.0077x; 1.0077x over previous
"""Trainium2 Bass kernel for nn_Explainer segment_reduce (cdist + bidirectional
segment max/mean) on 8 NeuronCores.

Math (reference):
    ef_n = (h[ne0] + h[ne1])/2, ef_l = (h[le0] + h[le1])/2
    M = -cdist(ef_n, ef_l)                      # [En, El]
    out_n = seg_mean_rows(seg_max_cols(M))      # [Gn, Gl]
    out_l = seg_mean_cols(seg_max_rows(M))      # [Gn, Gl]
    out = (out_n + out_l)/2

Device computes strip = 2*u_n.u_l - |u_l|^2 - |u_n|^2 = -|u_n - u_l|^2 = -4d^2:
the dot term via fp8e4 DoubleRow matmuls (K=256 in one instruction), -|u_l|^2
via a K=1 ones x (-bl2) fp16 matmul (spread over PE quadrant rows), -|u_n|^2
as the per-partition ACT bias during the psum->fp16 copy. All segment
reductions are then plain MAX; host maps back via d = 0.5*sqrt(-v).

Sharding: core c owns node segments [8c, 8c+8) in per-segment lane bands
(segment s -> lanes [B_s, B_s+L_s), row-tiles t in [0, nrt)); dummy slots
duplicate the segment's first row (can't win a max, no masking needed).
Label columns replicated; each segment padded to a multiple of W=8 with
duplicate edges (can't win a max either).

Per tile: PE fills psum groups; ACT converts fp32 psum -> fp16 strip with the
an2 bias; DVE does the W-block max tree (row side -> [P, B] block maxes) and
the running col max across tiles (col side). Host: fold blocks per label
segment + sqrt + means; band-collapse the col accumulator + sqrt + masked
means; assemble [64, 64].
"""
import numpy as np

import concourse.bacc as bacc
import concourse.tile as tile
import concourse.mybir as mybir
from concourse.bass_utils import run_bass_kernel_spmd

P = 128
N_CORES = 8
GN = GL = 64
D = 256
W = 8                      # label block width for the row-side max tree
F16 = mybir.dt.float16
F32 = mybir.dt.float32
F8 = mybir.dt.float8e4

_prog_cache = {}


def _groups(C):
    """psum column groups of 2048 (4 banks) + optional 512/1024/1536 rem."""
    gs = [(i * 2048, 2048) for i in range(C // 2048)]
    if C % 2048:
        gs.append((C - C % 2048, C % 2048))
    return gs


def _build(nrt: int, C: int, use_fp8: bool = False, C_real: int = 0):
    B = C // W
    C_real = C_real or C
    B_real = C_real // W
    groups = _groups(C)
    FD = F8 if use_fp8 else F16

    nc = bacc.Bacc("TRN2", target_bir_lowering=False, debug=False,
                   num_devices=N_CORES)
    ulT_in = nc.dram_tensor("ulT", [P, 2 * C], FD, kind="ExternalInput")
    unT_in = nc.dram_tensor("unT", [P, nrt * 2 * P], FD, kind="ExternalInput")
    bl2_in = nc.dram_tensor("bl2c", [4, C], F16, kind="ExternalInput")
    an2_in = nc.dram_tensor("an2q", [P, nrt], F32, kind="ExternalInput")
    rowout = nc.dram_tensor("rowout", [P, nrt * B], F16, kind="ExternalOutput")
    collout = nc.dram_tensor("collout", [P, C], F16, kind="ExternalOutput")

    with tile.TileContext(nc) as tc:
        with (
            tc.tile_pool(name="persist", bufs=1) as pp,
            tc.tile_pool(name="strip", bufs=3) as sp,
            tc.tile_pool(name="s1", bufs=2) as s1p,
            tc.tile_pool(name="row", bufs=3) as rp,
        ):
            u_lT = [pp.tile([P, 2, w], FD, tag=f"u_lT{gi}",
                            name=f"u_lT{gi}")
                    for gi, (g0, w) in enumerate(groups)]
            u_nT = pp.tile([P, nrt, 2, P], FD, tag="u_nT")
            bl2rep = pp.tile([P, C], F16, tag="bl2rep")
            onesrep = pp.tile([P, P], F16, tag="onesrep")
            an2q = pp.tile([P, nrt], F32, tag="an2q")
            coll = pp.tile([P, C], F16, tag="coll")

            nc.gpsimd.memset(onesrep[:], 1.0)
            # issue order: tile-0/group-0 dependencies only; remaining label
            # groups are prefetched inside the tile-0 loop (the DMA sem is
            # cumulative per queue, so anything emitted before the first
            # matmul delays it)
            nc.sync.dma_start(u_nT[:].rearrange("p t k q -> p (t k q)"),
                              unT_in[:])
            g0, w = groups[0]
            for k in range(2):
                nc.sync.dma_start(u_lT[0][:, k, :],
                                  ulT_in[:, k * C + g0:k * C + g0 + w])
            nc.sync.dma_start(bl2rep[0:97:32, :], bl2_in[:])
            nc.sync.dma_start(an2q[:], an2_in[:])

            collv = coll[:].rearrange("p (b w) -> p b w", w=W)
            with tc.tile_pool(name="ps", bufs=2, space="PSUM") as pg:
                for t in range(nrt):
                    strip = sp.tile([P, B, W], F16, tag="strip")
                    stripf = strip[:].rearrange("p b w -> p (b w)")
                    for gi, (g0, w) in enumerate(groups):
                        if t == 0 and gi + 1 < len(groups):
                            # prefetch next label group during this one's work
                            gn, wn = groups[gi + 1]
                            for k in range(2):
                                nc.sync.dma_start(
                                    u_lT[gi + 1][:, k, :],
                                    ulT_in[:, k * C + gn:k * C + gn + wn])
                        ptf = pg.tile([P, 2048], F32, tag="dot")
                        pt = ptf[:, :w]
                        nchunk = w // 512
                        if use_fp8:
                            for j in range(nchunk):
                                nc.tensor.matmul(
                                    pt[:, j * 512:(j + 1) * 512],
                                    u_nT[:, t, :, :],
                                    u_lT[gi][:, :, j * 512:(j + 1) * 512],
                                    start=True, stop=False,
                                    perf_mode=mybir.MatmulPerfMode.DoubleRow)
                        else:
                            for k in range(2):
                                for j in range(nchunk):
                                    nc.tensor.matmul(
                                        pt[:, j * 512:(j + 1) * 512],
                                        u_nT[:, t, k, :],
                                        u_lT[gi][:, k, j * 512:(j + 1) * 512],
                                        start=(k == 0), stop=False)
                        for j in range(nchunk):
                            pb = 32 * (j % 4)
                            nc.tensor.matmul(
                                pt[:, j * 512:(j + 1) * 512],
                                onesrep[pb:pb + 1, :],
                                bl2rep[pb:pb + 1,
                                       g0 + j * 512:g0 + (j + 1) * 512],
                                start=False, stop=True,
                                tile_position=(pb, 0))
                        dst = coll[:, g0:g0 + w] if t == 0 else stripf[:, g0:g0 + w]
                        nc.scalar.activation(
                            dst, pt[:], mybir.ActivationFunctionType.Identity,
                            bias=an2q[:, t:t + 1], scale=1.0)
                        # col side: running max, emitted per group so DVE
                        # tracks the conversions (t=0 wrote coll directly)
                        we = min(w, C_real - g0)       # skip pad-only cols
                        if t > 0 and we > 0:
                            nc.vector.tensor_max(coll[:, g0:g0 + we],
                                                 coll[:, g0:g0 + we],
                                                 stripf[:, g0:g0 + we])
                            if t == nrt - 1:
                                nc.sync.dma_start(collout[:, g0:g0 + we],
                                                  coll[:, g0:g0 + we])
                    # row side: W-block max tree -> [P, B_real]
                    src = collv if t == 0 else strip[:]
                    s1 = s1p.tile([P, B_real, 4], F16, tag="s1")
                    nc.vector.tensor_max(s1[:], src[:, :B_real, 0:4],
                                         src[:, :B_real, 4:8])
                    nc.vector.tensor_max(s1[:, :, 0:2], s1[:, :, 0:2],
                                         s1[:, :, 2:4])
                    rst = rp.tile([P, B_real], F16, tag="rst")
                    nc.vector.tensor_max(rst[:], s1[:, :, 0], s1[:, :, 1])
                    nc.sync.dma_start(rowout[:, t * B:t * B + B_real], rst[:])

    nc.compile()
    return nc


def _get_program(nrt, C, use_fp8, C_real):
    key = (nrt, C, use_fp8, C_real)
    if key not in _prog_cache:
        _prog_cache[key] = _build(nrt, C, use_fp8, C_real)
    return _prog_cache[key]


def _band_layout(sizes, nrt):
    """Lane bands: segment s gets L_s = ceil(size_s/nrt) lanes."""
    L = [-(-int(s) // nrt) if s > 0 else 0 for s in sizes]
    B = np.concatenate([[0], np.cumsum(L)]).astype(np.int64)
    return B, L


def kernel(h, node_edge, node_batch, label_edge, label_batch):
    h = np.asarray(h)
    ne = np.asarray(node_edge).astype(np.int64)
    nb = np.asarray(node_batch).astype(np.int64)
    le = np.asarray(label_edge).astype(np.int64)
    lb = np.asarray(label_batch).astype(np.int64)
    use_fp8 = True
    fdt = mybir.dt.np(F8 if use_fp8 else F16)

    cn = np.bincount(nb, minlength=GN).astype(np.int64)
    cl = np.bincount(lb, minlength=GL).astype(np.int64)
    nb_off = np.concatenate([[0], np.cumsum(cn)])
    lb_off = np.concatenate([[0], np.cumsum(cl)])

    # ---- label columns: each segment padded to a multiple of W with
    # duplicate edges; then global pad to a multiple of 512 with col 0 dups
    bg = -(-cl // W)                       # blocks per segment
    b_off = np.concatenate([[0], np.cumsum(bg)])
    B_real = int(b_off[-1])
    C = -(-(B_real * W) // 512) * 512
    B = C // W

    col_edge = np.zeros(C, np.int64)
    for g in range(GL):
        n_g = int(cl[g])
        if n_g == 0:
            continue
        width = int(bg[g]) * W
        k = np.arange(width)
        col_edge[b_off[g] * W + k] = lb_off[g] + k % n_g

    hf = h.astype(np.float32)
    u_l = hf[le[0][col_edge]] + hf[le[1][col_edge]]            # [C, 256] fp32
    bq = u_l.astype(fdt)                                       # quantized b
    bl2 = (bq.astype(np.float32) ** 2).sum(axis=1)             # |b|^2
    ulT = np.ascontiguousarray(
        bq.T.reshape(2, P, C).transpose(1, 0, 2).reshape(P, 2 * C))
    bl2_rep = np.ascontiguousarray(np.broadcast_to(
        (-bl2).astype(np.float16)[None, :], (4, C)))

    # ---- node rows: per-core lane bands over 8 segments; dummy slots
    # duplicate the segment's first row
    core_sizes = cn.reshape(N_CORES, 8)
    nrt = max(1, int(-(-core_sizes.sum(1).max() // P)))
    while max(sum(-(-int(s) // nrt) for s in core_sizes[c] if s > 0)
              for c in range(N_CORES)) > P:
        nrt += 1
    nrows = nrt * P

    in_maps = []
    band_info = []
    for c in range(N_CORES):
        Bo, L = _band_layout(core_sizes[c], nrt)
        assert Bo[-1] <= P
        slot = np.zeros(nrows, np.int64)
        # fallback row for fully-unused lanes (any valid index)
        slot[:] = min(int(nb_off[8 * c]), ne.shape[1] - 1)
        for s in range(8):
            g = 8 * c + s
            n_g = int(cn[g])
            if n_g == 0:
                continue
            lanes_all = np.arange(Bo[s], Bo[s + 1])
            for tt in range(nrt):
                slot[tt * P + lanes_all] = nb_off[g]   # seg dup default
            j = np.arange(n_g)
            lanes = Bo[s] + j // nrt
            ts = j % nrt
            slot[ts * P + lanes] = nb_off[g] + j
        u_n = hf[ne[0][slot]] + hf[ne[1][slot]]                 # [nrows, 256]
        aq = (2.0 * u_n).astype(fdt)                            # quantized a
        an2 = ((aq.astype(np.float32) ** 2).sum(axis=1) * 0.25)
        # unT layout: [p(K%128), t, k, row] ; row r of tile t = aq[t*P + r]
        a = aq.reshape(nrt, P, 2, P)         # [t, row, k, p]
        unT = np.ascontiguousarray(a.transpose(3, 0, 2, 1).reshape(P, -1))
        an2q = np.ascontiguousarray((-an2).astype(np.float32)
                                    .reshape(nrt, P).T)
        in_maps.append({
            "ulT": ulT,
            "unT": unT,
            "bl2c": bl2_rep,
            "an2q": an2q,
        })
        band_info.append((Bo, L))

    nc = _get_program(nrt, C, use_fp8, B_real * W)
    res = run_bass_kernel_spmd(nc, in_maps, core_ids=list(range(N_CORES)))

    # ---- host unpack -----------------------------------------------------
    out_n = np.zeros((GN, GL), np.float64)
    out_l = np.zeros((GN, GL), np.float64)
    col_w = np.zeros(C, np.float64)
    for g in range(GL):
        col_w[b_off[g] * W:b_off[g] * W + int(cl[g])] = 1.0
    ridx = (b_off[:-1]).clip(0, max(B_real - 1, 0))
    cidx = (b_off[:-1] * W).clip(0, max(B_real * W - 1, 0))
    for c in range(N_CORES):
        r = res.results[c]
        rowe = r["rowout"].astype(np.float64).reshape(P, nrt, B)
        colle = r["collout"].astype(np.float64)                 # [128, C]
        Bo, L = band_info[c]
        for s in range(8):
            g = 8 * c + s
            n_g = int(cn[g])
            if n_g == 0:
                continue
            j = np.arange(n_g)
            lanes = Bo[s] + j // nrt
            ts = j % nrt
            blk = rowe[lanes, ts, :]                            # [n_g, B]
            segmax = np.maximum.reduceat(blk[:, :B_real], ridx, axis=1)
            d = 0.5 * np.sqrt(np.maximum(-segmax, 0.0))
            row_mean = -d.mean(axis=0)
            row_mean[cl == 0] = 0.0
            out_n[g] = row_mean

            ecol = colle[Bo[s]:Bo[s] + L[s], :].max(axis=0)     # [C]
            dcol = 0.5 * np.sqrt(np.maximum(-ecol, 0.0))
            sums = np.add.reduceat((dcol * col_w)[:B_real * W], cidx)
            col_mean = -(sums / np.maximum(cl, 1))
            col_mean[cl == 0] = 0.0
            out_l[g] = col_mean

    return ((out_n + out_l) * 0.5).astype(np.float32)
